# revision 13
# baseline (speedup 1.0000x reference)
"""Multi-head attention + residual + LayerNorm on 8 Trainium2 NeuronCores.

Problem: nn_MultiHeadAttention_446676599424
  B=2, S=2048, D_MODEL=1024, N_HEAD=16, D_K=64
  reference returns (x, attn_weights):
    x [B, S, D]  = LayerNorm(attn_out @ W_O.T + Q)
    attn_weights [B, H, S, S] = softmax(q k^T / sqrt(d_k))

Sharding: tensor-parallel over heads — 2 heads per core. Each core:
  1. projects q/k/v for its 2 heads (inputs pre-transposed on host so the
     contraction dim lands on SBUF partitions),
  2. computes scores TRANSPOSED  s_T[k, q] = k_proj q_proj^T (so that the
     softmax denominator falls out of the attn@v matmul as a ones-column
     of the stationary operand, and attn@v needs no transposes at all),
  3. exp via ScalarE (scale 1/8 fused), denominator = row 64 of the
     context matmul output, normalize with the replicated reciprocal,
  4. writes attn in [b, h, k, q] layout (host transposes back to [.., q, k]),
  5. AllToAll redistributes the per-head context [d, r] to row shards,
  6. every core computes its 512-row shard of context @ W_O.T + residual
     + LayerNorm and returns it; host concatenates.
"""

import sys

if "/opt/trn_rl_repo" not in sys.path:
    sys.path.insert(0, "/opt/trn_rl_repo")

import numpy as np

import concourse.bacc as bacc
import concourse.mybir as mybir
from concourse import tile

F32 = mybir.dt.float32
AF = mybir.ActivationFunctionType
ALU = mybir.AluOpType

B = 2
D = 1024
N_HEAD = 16
D_K = 64
N_CORES = 8
H_PER_CORE = N_HEAD // N_CORES  # 2
HD = H_PER_CORE * D_K  # 128, per-core head-dim block
I_TILES = D // 128  # 8 contraction tiles for the projections

# Dtype of everything that feeds the PE: float32r runs the array at full
# rate (fp32 storage, reduced-precision multiply); float32 is the exact
# (4x slower) path. numpy sees float32 either way.
MMDT = mybir.dt.float32r

# When True, build_nc adds DRAM taps of intermediates for debugging.
DEBUG_TAPS = False


def build_nc(S=2048):
    """Build the per-core Bass graph (same graph on all 8 cores)."""
    R = B * S  # total rows
    SHARD = R // N_CORES  # output row-shard per core
    KT = S // 128  # k-position tiles per batch
    QW = S // 2  # q-half width
    NQC = max(1, QW // 512)  # 512-col chunks per q-half
    QC = QW // NQC
    RC = max(1, R // 512)  # 512-col chunks of r for projections
    RCW = R // RC
    VT_PER_RC = RCW // 128  # v r-tiles per r-chunk
    R_TILES = R // 128

    nc = bacc.Bacc("TRN2", target_bir_lowering=False, debug=False,
                   num_devices=N_CORES)

    qT = nc.dram_tensor("qT", [D, R], MMDT, kind="ExternalInput")
    kT = nc.dram_tensor("kT", [D, R], MMDT, kind="ExternalInput")
    vT = nc.dram_tensor("vT", [D, R], MMDT, kind="ExternalInput")
    wqT = nc.dram_tensor("wqT", [D, HD], MMDT, kind="ExternalInput")
    wkT = nc.dram_tensor("wkT", [D, HD], MMDT, kind="ExternalInput")
    wvT = nc.dram_tensor("wvT", [D, HD], MMDT, kind="ExternalInput")
    woT = nc.dram_tensor("woT", [D, D], MMDT, kind="ExternalInput")
    q_res = nc.dram_tensor("q_res", [SHARD, D], F32, kind="ExternalInput")
    gamma = nc.dram_tensor("gamma", [1, D], MMDT, kind="ExternalInput")
    beta = nc.dram_tensor("beta", [1, D], MMDT, kind="ExternalInput")

    attn_out = nc.dram_tensor("attn_out", [B * H_PER_CORE, S, S], F32,
                              kind="ExternalOutput")
    x_out = nc.dram_tensor("x_out", [SHARD, D], F32, kind="ExternalOutput")
    if DEBUG_TAPS:
        dbg_qp = nc.dram_tensor("dbg_qp", [128, R], F32, kind="ExternalOutput")
        dbg_kp = nc.dram_tensor("dbg_kp", [128, R], F32, kind="ExternalOutput")
        dbg_v0 = nc.dram_tensor("dbg_v0", [128, (R // 128) * 96], F32,
                                kind="ExternalOutput")
        dbg_at = nc.dram_tensor("dbg_at", [128, S // 2], F32,
                                kind="ExternalOutput")
        dbg_den = nc.dram_tensor("dbg_den", [128, S // 2], F32,
                                 kind="ExternalOutput")
        dbg_ctxf = nc.dram_tensor("dbg_ctxf", [128, I_TILES * (R // N_CORES)],
                                  F32, kind="ExternalOutput")
        dbg_psc = nc.dram_tensor("dbg_psc", [128, S // 2], F32,
                                 kind="ExternalOutput")
        dbg_recip = nc.dram_tensor("dbg_recip", [128, S // 2], F32,
                                   kind="ExternalOutput")

    with tile.TileContext(nc) as tc:
        with (
            tc.tile_pool(name="persist", bufs=1) as pp,
            tc.tile_pool(name="dram", bufs=1, space="DRAM") as dp,
        ):
            # persistent SBUF state
            qp_sb = pp.tile([128, R], MMDT, tag="qp")   # q_proj^T [hd, r]
            kp_sb = pp.tile([128, R], MMDT, tag="kp")   # k_proj^T [hd, r]
            # v slots [k-tile, 96]: cols 0..63 = v, col 64 = 1.0 (cols
            # 65..95 pad to a 32-multiple M so the PE writes all rows)
            v_sb = [pp.tile([128, R_TILES * 96], MMDT, tag=f"v{h}", name=f"v_sb{h}")
                    for h in range(H_PER_CORE)]
            # normalized context^T per head [d_k, r]
            ctxn = [pp.tile([64, R], MMDT, tag=f"ctxn{h}", name=f"ctxn{h}")
                    for h in range(H_PER_CORE)]
            gamma_rep = pp.tile([128, D], F32, tag="gamma_rep")
            beta_rep = pp.tile([128, D], F32, tag="beta_rep")
            ones_sb = pp.tile([128, 128], MMDT, tag="ones_sb")

            a2a_in = dp.tile([N_CORES * HD, SHARD], MMDT, tag="a2a_in")
            a2a_out = dp.tile([N_CORES * HD, SHARD], MMDT, tag="a2a_out")

            # ---- phase 0: constants ----
            nc.vector.memset(v_sb[0][:, :].bitcast(F32), 1.0)
            nc.vector.memset(v_sb[1][:, :].bitcast(F32), 1.0)
            nc.vector.memset(ones_sb[:, :].bitcast(F32), 1.0)
            gb_sb = pp.tile([1, D], MMDT, tag="gb")
            bb_sb = pp.tile([1, D], MMDT, tag="bb")
            nc.sync.dma_start(out=gb_sb[:, :], in_=gamma[:, :])
            nc.sync.dma_start(out=bb_sb[:, :], in_=beta[:, :])
            with tc.tile_pool(name="ps0", bufs=1, space="PSUM") as ps0:
                for rep, row in ((gamma_rep, gb_sb), (beta_rep, bb_sb)):
                    psb = ps0.tile([128, D], F32, tag="psb",
                                   name=f"psb_{row.name}")
                    for jc in range(D // 512):
                        js = slice(512 * jc, 512 * (jc + 1))
                        nc.tensor.matmul(psb[:, js], ones_sb[0:1, :],
                                         row[0:1, js], start=True, stop=True)
                    nc.vector.tensor_copy(rep[:, :], psb[:, :])

            # ---- phase 1: projections ----
            with (
                tc.tile_pool(name="p1", bufs=3) as p1,
                tc.tile_pool(name="p1w", bufs=1) as p1w,
                tc.tile_pool(name="ps_qk", bufs=2, space="PSUM") as ps_qk,
                tc.tile_pool(name="ps_v", bufs=4, space="PSUM") as ps_v,
            ):
                wq_sb = p1w.tile([128, D], MMDT, tag="wq")
                wk_sb = p1w.tile([128, D], MMDT, tag="wk")
                wv_sb = p1w.tile([128, D], MMDT, tag="wv")
                for t in range(I_TILES):
                    c = slice(128 * t, 128 * (t + 1))
                    nc.sync.dma_start(out=wq_sb[:, c], in_=wqT[c, :])
                    nc.sync.dma_start(out=wk_sb[:, c], in_=wkT[c, :])
                    nc.sync.dma_start(out=wv_sb[:, c], in_=wvT[c, :])

                for rc in range(RC):
                    rs = slice(RCW * rc, RCW * (rc + 1))
                    psq = ps_qk.tile([128, RCW], F32, tag="psq")
                    psk = ps_qk.tile([128, RCW], F32, tag="psk")
                    vt_blocks = []
                    for it in range(I_TILES):
                        ic = slice(128 * it, 128 * (it + 1))
                        first, last = it == 0, it == I_TILES - 1
                        qt_b = p1.tile([128, RCW], MMDT, tag="qt")
                        kt_b = p1.tile([128, RCW], MMDT, tag="kt")
                        vt_b = p1.tile([128, RCW], MMDT, tag="vt",
                                       bufs=I_TILES + 1)
                        nc.sync.dma_start(out=qt_b[:, :], in_=qT[ic, rs])
                        nc.sync.dma_start(out=kt_b[:, :], in_=kT[ic, rs])
                        nc.sync.dma_start(out=vt_b[:, :], in_=vT[ic, rs])
                        vt_blocks.append(vt_b)
                        nc.tensor.matmul(psq[:, :], (wq_sb[:, ic]),
                                         (qt_b[:, :]),
                                         start=first, stop=last)
                        nc.tensor.matmul(psk[:, :], (wk_sb[:, ic]),
                                         (kt_b[:, :]),
                                         start=first, stop=last)
                    nc.vector.tensor_copy(qp_sb[:, rs], psq[:, :])
                    nc.vector.tensor_copy(kp_sb[:, rs], psk[:, :])
                    for t in range(VT_PER_RC):
                        psv = ps_v.tile([128, 128], F32, tag="psv",
                                        name=f"psv{rc}_{t}")
                        for it in range(I_TILES):
                            ic = slice(128 * it, 128 * (it + 1))
                            nc.tensor.matmul(
                                psv[:, :],
                                (vt_blocks[it][:, 128 * t:128 * (t + 1)]),
                                (wv_sb[:, ic]),
                                start=(it == 0), stop=(it == I_TILES - 1))
                        r_tile = rc * VT_PER_RC + t
                        for h in range(H_PER_CORE):
                            nc.vector.tensor_copy(
                                v_sb[h][:, 96 * r_tile:96 * r_tile + 64],
                                psv[:, 64 * h:64 * (h + 1)])

            # ---- phase 2: attention ----
            with (
                tc.tile_pool(name="p2a", bufs=1) as p2a,
                tc.tile_pool(name="p2w", bufs=4) as p2w,
                tc.tile_pool(name="p2r", bufs=2) as p2r,
                tc.tile_pool(name="ps_s", bufs=2, space="PSUM") as ps_s,
                tc.tile_pool(name="ps_c", bufs=1, space="PSUM") as ps_c,
                tc.tile_pool(name="ps_b", bufs=1, space="PSUM") as ps_b,
            ):
                for b in range(B):
                    for h in range(H_PER_CORE):
                        hs = slice(64 * h, 64 * (h + 1))
                        for qh in range(2):
                            q0 = S * b + QW * qh
                            at_tiles = []
                            psc = ps_c.tile([128, QW], F32, tag="psc")
                            for kt_i in range(KT):
                                ks = slice(S * b + 128 * kt_i,
                                           S * b + 128 * (kt_i + 1))
                                ps = ps_s.tile([128, QW], F32, tag="pss")
                                for qc in range(NQC):
                                    qs = slice(q0 + QC * qc,
                                               q0 + QC * (qc + 1))
                                    nc.tensor.matmul(
                                        ps[:, QC * qc:QC * (qc + 1)],
                                        (kp_sb[hs, ks]),
                                        (qp_sb[hs, qs]),
                                        start=True, stop=True)
                                at = p2a.tile([128, QW], MMDT,
                                              tag=f"attn{kt_i}")
                                nc.scalar.activation(at[:, :], ps[:, :],
                                                     AF.Exp, scale=0.125)
                                at_tiles.append(at)
                                slot = b * KT + kt_i
                                for qc in range(NQC):
                                    cs = slice(QC * qc, QC * (qc + 1))
                                    nc.tensor.matmul(
                                        psc[0:96, cs],
                                        (v_sb[h][:, 96 * slot:
                                                    96 * slot + 96]),
                                        (at[:, cs]),
                                        start=(kt_i == 0),
                                        stop=(kt_i == KT - 1))
                            if DEBUG_TAPS and b == 0 and h == 0 and qh == 0:
                                nc.sync.dma_start(
                                    out=dbg_at[:, :],
                                    in_=at_tiles[0][:, :].bitcast(F32))
                            # denominator -> replicated reciprocal:
                            # evict the ones-row, broadcast it across all
                            # partitions with a K=1 PE matmul, reciprocal.
                            den_sb = p2r.tile([128, QW], MMDT, tag="den_sb")
                            recip = p2r.tile([128, QW], F32, tag="recip")
                            psb = ps_b.tile([128, QW], F32, tag="psb")
                            nc.vector.tensor_copy(den_sb[64:65, :],
                                                  psc[64:65, :])
                            for qc in range(NQC):
                                cs = slice(QC * qc, QC * (qc + 1))
                                nc.tensor.matmul(psb[:, cs],
                                                 ones_sb[64:65, :],
                                                 den_sb[64:65, cs],
                                                 start=True, stop=True)
                            nc.vector.reciprocal_approx_fast(recip[:, :],
                                                             psb[:, :])
                            if DEBUG_TAPS and b == 0 and h == 0 and qh == 0:
                                nc.sync.dma_start(out=dbg_den[:, :],
                                                  in_=den_rep[:, :])
                                psc_tap = p2r.tile([128, QW], F32,
                                                   tag="den_rep",
                                                   name="psc_tap")
                                nc.vector.tensor_copy(psc_tap[0:96, :],
                                                      psc[0:96, :])
                                nc.sync.dma_start(out=dbg_psc[:, :],
                                                  in_=psc_tap[:, :])
                                nc.sync.dma_start(out=dbg_recip[:, :],
                                                  in_=recip[:, :])
                            # normalized context^T for this unit
                            nc.vector.tensor_mul(
                                ctxn[h][0:64, q0:q0 + QW],
                                psc[0:64, :], recip[0:64, :])
                            # normalize + write attention tiles
                            plane = b * H_PER_CORE + h
                            for kt_i in range(KT):
                                wt = p2w.tile([128, QW], F32, tag="wt")
                                nc.vector.tensor_mul(wt[:, :],
                                                     at_tiles[kt_i][:, :],
                                                     recip[:, :])
                                nc.sync.dma_start(
                                    out=attn_out[plane,
                                                 128 * kt_i:128 * (kt_i + 1),
                                                 QW * qh:QW * (qh + 1)],
                                    in_=wt[:, :])

            if DEBUG_TAPS:
                nc.sync.dma_start(out=dbg_qp[:, :],
                                  in_=qp_sb[:, :].bitcast(F32))
                nc.sync.dma_start(out=dbg_kp[:, :],
                                  in_=kp_sb[:, :].bitcast(F32))
                nc.sync.dma_start(out=dbg_v0[:, :],
                                  in_=v_sb[0][:, :].bitcast(F32))

            # ---- phase 3: all-to-all of the context ----
            for j in range(N_CORES):
                ss = slice(SHARD * j, SHARD * (j + 1))
                for h in range(H_PER_CORE):
                    nc.sync.dma_start(
                        out=a2a_in[HD * j + 64 * h:HD * j + 64 * (h + 1), :],
                        in_=ctxn[h][0:64, ss])
            nc.gpsimd.collective_compute(
                "AllToAll",
                ALU.bypass,
                replica_groups=[list(range(N_CORES))],
                ins=[a2a_in[:, :].opt()],
                outs=[a2a_out[:, :].opt()],
            )

            # ---- phase 4: W_O projection + residual + LayerNorm ----
            with (
                tc.tile_pool(name="p4", bufs=1) as p4,
                tc.tile_pool(name="p4x", bufs=2) as p4x,
                tc.tile_pool(name="ps_o", bufs=2, space="PSUM") as ps_o,
            ):
                wo_sb = p4.tile([128, I_TILES * D], MMDT, tag="wo")
                for t in range(I_TILES):
                    nc.sync.dma_start(
                        out=wo_sb[:, D * t:D * (t + 1)],
                        in_=woT[128 * t:128 * (t + 1), :])
                ctxf = p4.tile([128, I_TILES * SHARD], MMDT, tag="ctxf")
                for t in range(I_TILES):
                    nc.sync.dma_start(
                        out=ctxf[:, SHARD * t:SHARD * (t + 1)],
                        in_=a2a_out[128 * t:128 * (t + 1), :])

                if DEBUG_TAPS:
                    nc.sync.dma_start(out=dbg_ctxf[:, :],
                                      in_=ctxf[:, :].bitcast(F32))
                n_rt = (SHARD + 127) // 128
                for rt in range(n_rt):
                    rp = min(128, SHARD - 128 * rt)
                    rsl = slice(128 * rt, 128 * rt + rp)
                    xsb = p4x.tile([128, D], F32, tag="xsb")
                    qres_t = p4x.tile([128, D], F32, tag="qres")
                    nc.sync.dma_start(out=qres_t[:rp, :], in_=q_res[rsl, :])
                    for jc in range(2):
                        js = slice(512 * jc, 512 * (jc + 1))
                        pso = ps_o.tile([128, 512], F32, tag="pso")
                        for t in range(I_TILES):
                            nc.tensor.matmul(
                                pso[:rp, :],
                                (ctxf[:, SHARD * t + 128 * rt:
                                         SHARD * t + 128 * rt + rp]),
                                (wo_sb[:, D * t + 512 * jc:
                                          D * t + 512 * (jc + 1)]),
                                start=(t == 0), stop=(t == I_TILES - 1))
                        nc.vector.tensor_add(xsb[:rp, js], pso[:rp, :],
                                             qres_t[:rp, js])
                    # LayerNorm over the free dim (D)
                    mu = p4x.tile([128, 1], F32, tag="mu")
                    var = p4x.tile([128, 1], F32, tag="var")
                    rstd = p4x.tile([128, 1], F32, tag="rstd")
                    xc = p4x.tile([128, D], F32, tag="xc")
                    sq = p4x.tile([128, D], F32, tag="sq")
                    xo = p4x.tile([128, D], F32, tag="xo")
                    nc.vector.tensor_reduce(mu[:rp, :], xsb[:rp, :],
                                            axis=mybir.AxisListType.X,
                                            op=ALU.add)
                    nc.vector.tensor_scalar_mul(mu[:rp, :], mu[:rp, :],
                                                1.0 / D)
                    nc.vector.tensor_scalar(xc[:rp, :], xsb[:rp, :],
                                            mu[:rp, :], None,
                                            op0=ALU.subtract)
                    nc.vector.scalar_tensor_tensor(
                        sq[:rp, :], in0=xc[:rp, :], scalar=1.0,
                        in1=xc[:rp, :], op0=ALU.mult, op1=ALU.mult,
                        accum_out=var[:rp, :])
                    nc.vector.tensor_scalar(var[:rp, :], var[:rp, :],
                                            1.0 / D, 1e-5,
                                            op0=ALU.mult, op1=ALU.add)
                    # rstd = exp(-0.5 * ln(var+eps)) (Log/Exp share a table)
                    nc.scalar.activation(rstd[:rp, :], var[:rp, :], AF.Ln)
                    nc.scalar.activation(rstd[:rp, :], rstd[:rp, :], AF.Exp,
                                         scale=-0.5)
                    nc.vector.scalar_tensor_tensor(
                        xo[:rp, :], in0=xc[:rp, :], scalar=rstd[:rp, :],
                        in1=gamma_rep[:rp, :], op0=ALU.mult, op1=ALU.mult)
                    nc.vector.tensor_add(xo[:rp, :], xo[:rp, :],
                                         beta_rep[:rp, :])
                    nc.sync.dma_start(out=x_out[rsl, :], in_=xo[:rp, :])

    nc.compile()
    return nc


_NC_CACHE = {}


def _get_nc(S):
    if S not in _NC_CACHE:
        _NC_CACHE[S] = build_nc(S)
    return _NC_CACHE[S]


def make_in_maps(Q, K, V, W_Q, W_K, W_V, W_O, ln_gamma, ln_beta):
    S = Q.shape[1]
    R = B * S
    SHARD = R // N_CORES
    Q2 = np.asarray(Q, np.float32).reshape(R, D)
    QT = np.ascontiguousarray(Q2.T)
    KT_ = np.ascontiguousarray(np.asarray(K, np.float32).reshape(R, D).T)
    VT_ = np.ascontiguousarray(np.asarray(V, np.float32).reshape(R, D).T)
    woT = np.ascontiguousarray(np.asarray(W_O, np.float32).T)
    g = np.asarray(ln_gamma, np.float32).reshape(1, D)
    bta = np.asarray(ln_beta, np.float32).reshape(1, D)
    in_maps = []
    for c in range(N_CORES):
        hsl = slice(HD * c, HD * (c + 1))
        in_maps.append({
            "qT": QT, "kT": KT_, "vT": VT_,
            "wqT": np.ascontiguousarray(np.asarray(W_Q, np.float32)[hsl, :].T),
            "wkT": np.ascontiguousarray(np.asarray(W_K, np.float32)[hsl, :].T),
            "wvT": np.ascontiguousarray(np.asarray(W_V, np.float32)[hsl, :].T),
            "woT": woT,
            "q_res": np.ascontiguousarray(Q2[SHARD * c:SHARD * (c + 1), :]),
            "gamma": g, "beta": bta,
        })
    return in_maps


def assemble(results, S):
    R = B * S
    SHARD = R // N_CORES
    x = np.empty((R, D), np.float32)
    attn = np.empty((B, N_HEAD, S, S), np.float32)
    for c in range(N_CORES):
        x[SHARD * c:SHARD * (c + 1), :] = results[c]["x_out"]
        a = results[c]["attn_out"]  # [B*H_PER_CORE, S(k), S(q)]
        for b in range(B):
            for h in range(H_PER_CORE):
                attn[b, H_PER_CORE * c + h] = a[b * H_PER_CORE + h].T
    return x.reshape(B, S, D), attn


def kernel(Q, K, V, W_Q, W_K, W_V, W_O, ln_gamma, ln_beta):
    from concourse.bass_utils import run_bass_kernel_spmd

    S = Q.shape[1]
    nc = _get_nc(S)
    in_maps = make_in_maps(Q, K, V, W_Q, W_K, W_V, W_O, ln_gamma, ln_beta)
    res = run_bass_kernel_spmd(nc, in_maps, core_ids=list(range(N_CORES)))
    return assemble(res.results, S)


# revision 15
# speedup vs baseline: 1.2680x; 1.2680x over previous
"""Multi-head attention + residual + LayerNorm on 8 Trainium2 NeuronCores.

Problem: nn_MultiHeadAttention_446676599424
  B=2, S=2048, D_MODEL=1024, N_HEAD=16, D_K=64
  reference returns (x, attn_weights):
    x [B, S, D]  = LayerNorm(attn_out @ W_O.T + Q)
    attn_weights [B, H, S, S] = softmax(q k^T / sqrt(d_k))

Sharding: tensor-parallel over heads — 2 heads per core. Each core:
  1. projects q/k/v for its 2 heads (inputs pre-transposed on host so the
     contraction dim lands on SBUF partitions); q/k kept in float32r for
     score precision, v cast to bf16,
  2. computes scores TRANSPOSED  s_T[k, q] = k_proj q_proj^T (so the
     softmax denominator falls out of the attn@v matmul as a ones-column
     of the stationary operand and attn@v needs no transposes),
  3. exp via ScalarE (scale 1/8 fused) into bf16 tiles, denominator =
     row 64 of the context matmul output, replicated across partitions
     with a K=1 PE matmul, reciprocal on DVE,
  4. writes attn in [b, h, k, q] bf16 (host transposes/upcasts),
  5. AllToAll redistributes the per-head context [d, r] to row shards,
  6. every core computes its 512-row shard of context @ W_O.T + residual
     + LayerNorm (fp32) and returns it; host concatenates.
"""

import sys

if "/opt/trn_rl_repo" not in sys.path:
    sys.path.insert(0, "/opt/trn_rl_repo")

import numpy as np

import concourse.bacc as bacc
import concourse.mybir as mybir
from concourse import tile

F32 = mybir.dt.float32
BF16 = mybir.dt.bfloat16
AF = mybir.ActivationFunctionType
ALU = mybir.AluOpType

B = 2
D = 1024
N_HEAD = 16
D_K = 64
N_CORES = 8
H_PER_CORE = N_HEAD // N_CORES  # 2
HD = H_PER_CORE * D_K  # 128, per-core head-dim block
I_TILES = D // 128  # 8 contraction tiles for the projections

# float32r: fp32 storage, full-rate PE with reduced multiply mantissa.
MMDT = mybir.dt.float32r


def build_nc(S=2048):
    """Build the per-core Bass graph (same graph on all 8 cores)."""
    R = B * S  # total rows
    SHARD = R // N_CORES  # output row-shard per core
    KT = S // 128  # k-position tiles per batch
    QW = S // 2  # q-half width (one attention work unit)
    NQC = max(1, QW // 512)
    QC = QW // NQC  # matmul N-chunk
    RC = max(1, R // 512)  # 512-col r-chunks for the projections
    RCW = R // RC
    VT_PER_RC = RCW // 128
    KT_B = KT  # v slots per (head, batch) tile

    nc = bacc.Bacc("TRN2", target_bir_lowering=False, debug=False,
                   num_devices=N_CORES)

    qT = nc.dram_tensor("qT", [D, R], MMDT, kind="ExternalInput")
    kT = nc.dram_tensor("kT", [D, R], MMDT, kind="ExternalInput")
    vT = nc.dram_tensor("vT", [D, R], MMDT, kind="ExternalInput")
    wqT = nc.dram_tensor("wqT", [D, HD], MMDT, kind="ExternalInput")
    wkT = nc.dram_tensor("wkT", [D, HD], MMDT, kind="ExternalInput")
    wvT = nc.dram_tensor("wvT", [D, HD], MMDT, kind="ExternalInput")
    woT = nc.dram_tensor("woT", [D, D], BF16, kind="ExternalInput")
    q_res = nc.dram_tensor("q_res", [SHARD, D], F32, kind="ExternalInput")
    gamma = nc.dram_tensor("gamma", [1, D], MMDT, kind="ExternalInput")
    beta = nc.dram_tensor("beta", [1, D], MMDT, kind="ExternalInput")

    attn_out = nc.dram_tensor("attn_out", [B * H_PER_CORE, S, S], BF16,
                              kind="ExternalOutput")
    x_out = nc.dram_tensor("x_out", [SHARD, D], F32, kind="ExternalOutput")

    with tile.TileContext(nc) as tc:
        with (
            tc.tile_pool(name="persist", bufs=1) as pp,
            tc.tile_pool(name="dram", bufs=1, space="DRAM") as dp,
        ):
            # chunked so phase 2 can start before all projections finish
            qp = [pp.tile([128, RCW], MMDT, tag=f"qp{rc}", name=f"qp{rc}")
                  for rc in range(RC)]
            kp = [pp.tile([128, RCW], MMDT, tag=f"kp{rc}", name=f"kp{rc}")
                  for rc in range(RC)]
            # v slots per (head, batch): [k-tile, 96] bf16
            # (cols 0..63 v, col 64 = 1.0, cols 65..95 pad for 32-mult M)
            v_sb = {(h, b): pp.tile([128, KT_B * 96], BF16,
                                    tag=f"v{h}{b}", name=f"v_sb{h}{b}")
                    for h in range(H_PER_CORE) for b in range(B)}
            # normalized context^T per head [d_k, r], bf16
            ctxn = [pp.tile([64, R], BF16, tag=f"ctxn{h}", name=f"ctxn{h}")
                    for h in range(H_PER_CORE)]
            gamma_rep = pp.tile([128, D], F32, tag="gamma_rep")
            beta_rep = pp.tile([128, D], F32, tag="beta_rep")
            ones_sb = pp.tile([128, 128], MMDT, tag="ones_sb")
            wo_sb = pp.tile([128, I_TILES * D], BF16, tag="wo")
            n_rt = (SHARD + 127) // 128
            qres_sb = pp.tile([128, n_rt * D], F32, tag="qres")

            a2a_in = dp.tile([N_CORES * HD, SHARD], BF16, tag="a2a_in")
            a2a_out = dp.tile([N_CORES * HD, SHARD], BF16, tag="a2a_out")

            # ---- phase 0: constants + weight/residual prefetch ----
            for t in v_sb.values():
                nc.vector.memset(t[:, :], 1.0)
            nc.vector.memset(ones_sb[:, :].bitcast(F32), 1.0)
            gb_sb = pp.tile([1, D], MMDT, tag="gb")
            bb_sb = pp.tile([1, D], MMDT, tag="bb")
            nc.sync.dma_start(out=gb_sb[:, :], in_=gamma[:, :])
            nc.sync.dma_start(out=bb_sb[:, :], in_=beta[:, :])
            for t in range(I_TILES):
                nc.sync.dma_start(out=wo_sb[:, D * t:D * (t + 1)],
                                  in_=woT[128 * t:128 * (t + 1), :])
            for t in range(n_rt):
                rp0 = min(128, SHARD - 128 * t)
                nc.sync.dma_start(
                    out=qres_sb[:rp0, D * t:D * (t + 1)],
                    in_=q_res[128 * t:128 * t + rp0, :])
            with tc.tile_pool(name="ps0", bufs=1, space="PSUM") as ps0:
                for rep, row in ((gamma_rep, gb_sb), (beta_rep, bb_sb)):
                    psb0 = ps0.tile([128, D], F32, tag="psb0",
                                    name=f"psb0_{rep.name}")
                    for jc in range(D // 512):
                        js = slice(512 * jc, 512 * (jc + 1))
                        nc.tensor.matmul(psb0[:, js], ones_sb[0:1, :],
                                         row[0:1, js], start=True, stop=True)
                    nc.vector.tensor_copy(rep[:, :], psb0[:, :])

            # ---- phase 1: projections ----
            with (
                tc.tile_pool(name="p1", bufs=3) as p1,
                tc.tile_pool(name="p1w", bufs=1) as p1w,
                tc.tile_pool(name="ps_qk", bufs=2, space="PSUM") as ps_qk,
                tc.tile_pool(name="ps_v", bufs=4, space="PSUM") as ps_v,
            ):
                wq_sb = p1w.tile([128, D], MMDT, tag="wq")
                wk_sb = p1w.tile([128, D], MMDT, tag="wk")
                wv_sb = p1w.tile([128, D], MMDT, tag="wv")
                for t in range(I_TILES):
                    c = slice(128 * t, 128 * (t + 1))
                    nc.sync.dma_start(out=wq_sb[:, c], in_=wqT[c, :])
                    nc.sync.dma_start(out=wk_sb[:, c], in_=wkT[c, :])
                    nc.sync.dma_start(out=wv_sb[:, c], in_=wvT[c, :])

                for rc in range(RC):
                    rs = slice(RCW * rc, RCW * (rc + 1))
                    psq = ps_qk.tile([128, RCW], F32, tag="psq")
                    psk = ps_qk.tile([128, RCW], F32, tag="psk")
                    vt_blocks = []
                    for it in range(I_TILES):
                        ic = slice(128 * it, 128 * (it + 1))
                        first, last = it == 0, it == I_TILES - 1
                        qt_b = p1.tile([128, RCW], MMDT, tag="qt")
                        kt_b = p1.tile([128, RCW], MMDT, tag="kt")
                        vt_b = p1.tile([128, RCW], MMDT, tag="vt",
                                       bufs=I_TILES + 1)
                        nc.sync.dma_start(out=qt_b[:, :], in_=qT[ic, rs])
                        nc.sync.dma_start(out=kt_b[:, :], in_=kT[ic, rs])
                        nc.sync.dma_start(out=vt_b[:, :], in_=vT[ic, rs])
                        vt_blocks.append(vt_b)
                        nc.tensor.matmul(psq[:, :], wq_sb[:, ic], qt_b[:, :],
                                         start=first, stop=last)
                        nc.tensor.matmul(psk[:, :], wk_sb[:, ic], kt_b[:, :],
                                         start=first, stop=last)
                    nc.vector.tensor_copy(qp[rc][:, :], psq[:, :])
                    nc.vector.tensor_copy(kp[rc][:, :], psk[:, :])
                    for t in range(VT_PER_RC):
                        psv = ps_v.tile([128, 128], F32, tag="psv",
                                        name=f"psv{rc}_{t}")
                        for it in range(I_TILES):
                            ic = slice(128 * it, 128 * (it + 1))
                            nc.tensor.matmul(
                                psv[:, :],
                                vt_blocks[it][:, 128 * t:128 * (t + 1)],
                                wv_sb[:, ic],
                                start=(it == 0), stop=(it == I_TILES - 1))
                        r_tile = rc * VT_PER_RC + t  # global r tile
                        b_ix, kt_ix = divmod(r_tile, KT_B)
                        for h in range(H_PER_CORE):
                            nc.vector.tensor_copy(
                                v_sb[(h, b_ix)][:, 96 * kt_ix:96 * kt_ix + 64],
                                psv[:, 64 * h:64 * (h + 1)])

            # ---- phase 2: attention ----
            def pkslice(col0, width):
                """(tile_index, slice) within the chunked qp/kp tiles."""
                t = col0 // RCW
                o = col0 - t * RCW
                assert o + width <= RCW
                return t, slice(o, o + width)

            with (
                tc.tile_pool(name="p2a", bufs=2) as p2a,
                tc.tile_pool(name="p2w", bufs=4) as p2w,
                tc.tile_pool(name="p2r", bufs=2) as p2r,
                tc.tile_pool(name="ps_s", bufs=2, space="PSUM") as ps_s,
                tc.tile_pool(name="ps_c", bufs=1, space="PSUM") as ps_c,
                tc.tile_pool(name="ps_b", bufs=1, space="PSUM") as ps_b,
            ):
                for b in range(B):
                    for h in range(H_PER_CORE):
                        hs = slice(64 * h, 64 * (h + 1))
                        for qh in range(2):
                            q0 = S * b + QW * qh
                            at_tiles = []
                            psc = ps_c.tile([128, QW], F32, tag="psc")
                            for kt_i in range(KT):
                                kt_t, kt_s = pkslice(S * b + 128 * kt_i, 128)
                                ps = ps_s.tile([128, QW], F32, tag="pss")
                                for qc in range(NQC):
                                    qt_t, qt_s = pkslice(q0 + QC * qc, QC)
                                    nc.tensor.matmul(
                                        ps[:, QC * qc:QC * (qc + 1)],
                                        kp[kt_t][hs, kt_s],
                                        qp[qt_t][hs, qt_s],
                                        start=True, stop=True)
                                at = p2a.tile([128, QW], BF16,
                                              tag=f"attn{kt_i}")
                                nc.scalar.activation(at[:, :], ps[:, :],
                                                     AF.Exp, scale=0.125)
                                at_tiles.append(at)
                                for qc in range(NQC):
                                    cs = slice(QC * qc, QC * (qc + 1))
                                    nc.tensor.matmul(
                                        psc[0:96, cs],
                                        v_sb[(h, b)][:, 96 * kt_i:
                                                     96 * kt_i + 96],
                                        at[:, cs],
                                        start=(kt_i == 0),
                                        stop=(kt_i == KT - 1))
                            # denominator -> replicated reciprocal
                            den_sb = p2r.tile([128, QW], MMDT, tag="den_sb")
                            recip = p2r.tile([128, QW], F32, tag="recip")
                            recip_b = p2r.tile([128, QW], BF16, tag="recip_b")
                            psb = ps_b.tile([128, QW], F32, tag="psb")
                            nc.vector.tensor_copy(den_sb[64:65, :],
                                                  psc[64:65, :])
                            for qc in range(NQC):
                                cs = slice(QC * qc, QC * (qc + 1))
                                nc.tensor.matmul(psb[:, cs],
                                                 ones_sb[64:65, :],
                                                 den_sb[64:65, cs],
                                                 start=True, stop=True)
                            nc.vector.reciprocal_approx_fast(recip[:, :],
                                                             psb[:, :])
                            nc.vector.tensor_copy(recip_b[:, :], recip[:, :])
                            # normalized context^T for this unit
                            nc.vector.tensor_mul(
                                ctxn[h][0:64, q0:q0 + QW],
                                psc[0:64, :], recip[0:64, :])
                            # normalize + write attention tiles
                            plane = b * H_PER_CORE + h
                            for kt_i in range(KT):
                                wt = p2w.tile([128, QW], BF16, tag="wt")
                                nc.vector.tensor_mul(wt[:, :],
                                                     at_tiles[kt_i][:, :],
                                                     recip_b[:, :])
                                nc.sync.dma_start(
                                    out=attn_out[plane,
                                                 128 * kt_i:128 * (kt_i + 1),
                                                 QW * qh:QW * (qh + 1)],
                                    in_=wt[:, :])

            # ---- phase 3: all-to-all of the context ----
            for j in range(N_CORES):
                ss = slice(SHARD * j, SHARD * (j + 1))
                for h in range(H_PER_CORE):
                    nc.sync.dma_start(
                        out=a2a_in[HD * j + 64 * h:HD * j + 64 * (h + 1), :],
                        in_=ctxn[h][0:64, ss])
            nc.gpsimd.collective_compute(
                "AllToAll",
                ALU.bypass,
                replica_groups=[list(range(N_CORES))],
                ins=[a2a_in[:, :].opt()],
                outs=[a2a_out[:, :].opt()],
            )

            # ---- phase 4: W_O projection + residual + LayerNorm ----
            with (
                tc.tile_pool(name="p4", bufs=1) as p4,
                tc.tile_pool(name="p4x", bufs=2) as p4x,
                tc.tile_pool(name="ps_o", bufs=2, space="PSUM") as ps_o,
            ):
                ctxf = p4.tile([128, I_TILES * SHARD], BF16, tag="ctxf")
                for t in range(I_TILES):
                    nc.sync.dma_start(
                        out=ctxf[:, SHARD * t:SHARD * (t + 1)],
                        in_=a2a_out[128 * t:128 * (t + 1), :])

                for rt in range(n_rt):
                    rp = min(128, SHARD - 128 * rt)
                    rsl = slice(128 * rt, 128 * rt + rp)
                    xsb = p4x.tile([128, D], F32, tag="xsb")
                    for jc in range(2):
                        js = slice(512 * jc, 512 * (jc + 1))
                        pso = ps_o.tile([128, 512], F32, tag="pso")
                        for t in range(I_TILES):
                            nc.tensor.matmul(
                                pso[:rp, :],
                                ctxf[:, SHARD * t + 128 * rt:
                                     SHARD * t + 128 * rt + rp],
                                wo_sb[:, D * t + 512 * jc:
                                      D * t + 512 * (jc + 1)],
                                start=(t == 0), stop=(t == I_TILES - 1))
                        nc.vector.tensor_add(
                            xsb[:rp, js], pso[:rp, :],
                            qres_sb[:rp, D * rt + 512 * jc:
                                    D * rt + 512 * (jc + 1)])
                    # LayerNorm over the free dim (D)
                    mu = p4x.tile([128, 1], F32, tag="mu")
                    var = p4x.tile([128, 1], F32, tag="var")
                    rstd = p4x.tile([128, 1], F32, tag="rstd")
                    xc = p4x.tile([128, D], F32, tag="xc")
                    sq = p4x.tile([128, D], F32, tag="sq")
                    xo = p4x.tile([128, D], F32, tag="xo")
                    nc.vector.tensor_reduce(mu[:rp, :], xsb[:rp, :],
                                            axis=mybir.AxisListType.X,
                                            op=ALU.add)
                    nc.vector.tensor_scalar_mul(mu[:rp, :], mu[:rp, :],
                                                1.0 / D)
                    nc.vector.tensor_scalar(xc[:rp, :], xsb[:rp, :],
                                            mu[:rp, :], None,
                                            op0=ALU.subtract)
                    nc.vector.scalar_tensor_tensor(
                        sq[:rp, :], in0=xc[:rp, :], scalar=1.0,
                        in1=xc[:rp, :], op0=ALU.mult, op1=ALU.mult,
                        accum_out=var[:rp, :])
                    nc.vector.tensor_scalar(var[:rp, :], var[:rp, :],
                                            1.0 / D, 1e-5,
                                            op0=ALU.mult, op1=ALU.add)
                    # rstd = exp(-0.5 * ln(var+eps)) (Ln/Exp share a table)
                    nc.scalar.activation(rstd[:rp, :], var[:rp, :], AF.Ln)
                    nc.scalar.activation(rstd[:rp, :], rstd[:rp, :], AF.Exp,
                                         scale=-0.5)
                    nc.vector.scalar_tensor_tensor(
                        xo[:rp, :], in0=xc[:rp, :], scalar=rstd[:rp, :],
                        in1=gamma_rep[:rp, :], op0=ALU.mult, op1=ALU.mult)
                    nc.vector.tensor_add(xo[:rp, :], xo[:rp, :],
                                         beta_rep[:rp, :])
                    nc.sync.dma_start(out=x_out[rsl, :], in_=xo[:rp, :])

    nc.compile()
    return nc


_NC_CACHE = {}


def _get_nc(S):
    if S not in _NC_CACHE:
        _NC_CACHE[S] = build_nc(S)
    return _NC_CACHE[S]


def make_in_maps(Q, K, V, W_Q, W_K, W_V, W_O, ln_gamma, ln_beta):
    S = Q.shape[1]
    R = B * S
    SHARD = R // N_CORES
    bf16 = mybir.dt.np(BF16)
    Q2 = np.asarray(Q, np.float32).reshape(R, D)
    QT = np.ascontiguousarray(Q2.T)
    KT_ = np.ascontiguousarray(np.asarray(K, np.float32).reshape(R, D).T)
    VT_ = np.ascontiguousarray(np.asarray(V, np.float32).reshape(R, D).T)
    woT = np.ascontiguousarray(np.asarray(W_O, np.float32).T).astype(bf16)
    g = np.asarray(ln_gamma, np.float32).reshape(1, D)
    bta = np.asarray(ln_beta, np.float32).reshape(1, D)
    in_maps = []
    for c in range(N_CORES):
        hsl = slice(HD * c, HD * (c + 1))
        in_maps.append({
            "qT": QT, "kT": KT_, "vT": VT_,
            "wqT": np.ascontiguousarray(np.asarray(W_Q, np.float32)[hsl, :].T),
            "wkT": np.ascontiguousarray(np.asarray(W_K, np.float32)[hsl, :].T),
            "wvT": np.ascontiguousarray(np.asarray(W_V, np.float32)[hsl, :].T),
            "woT": woT,
            "q_res": np.ascontiguousarray(Q2[SHARD * c:SHARD * (c + 1), :]),
            "gamma": g, "beta": bta,
        })
    return in_maps


def assemble(results, S):
    R = B * S
    SHARD = R // N_CORES
    x = np.empty((R, D), np.float32)
    attn = np.empty((B, N_HEAD, S, S), np.float32)
    for c in range(N_CORES):
        x[SHARD * c:SHARD * (c + 1), :] = results[c]["x_out"]
        a = np.asarray(results[c]["attn_out"], dtype=np.float32)
        for b in range(B):
            for h in range(H_PER_CORE):
                attn[b, H_PER_CORE * c + h] = a[b * H_PER_CORE + h].T
    return x.reshape(B, S, D), attn


def kernel(Q, K, V, W_Q, W_K, W_V, W_O, ln_gamma, ln_beta):
    from concourse.bass_utils import run_bass_kernel_spmd

    S = Q.shape[1]
    nc = _get_nc(S)
    in_maps = make_in_maps(Q, K, V, W_Q, W_K, W_V, W_O, ln_gamma, ln_beta)
    res = run_bass_kernel_spmd(nc, in_maps, core_ids=list(range(N_CORES)))
    return assemble(res.results, S)
